# revision 9
# baseline (speedup 1.0000x reference)
"""GCNConv Trainium2 kernel: out = (segment_sum(edge_val * X[edge_col], edge_row)) @ W.

Strategy (8-core SPMD, 1D destination-row sharding):
  - Host: edges are split by destination row across 8 cores. Since the fast
    SWDGE dma_gather uses int16 indices, X is replicated as 4 table chunks of
    32768 rows; each core's edges are grouped by (source chunk, dest row).
  - Per chunk, edges (sorted by dest row) are packed into "bins" of <=128
    edges spanning <=16 row-slots (rows may split across bins). Each bin is
    one PE matmul: lhsT = gathered messages [128 edges, 128 d] (dma_gather),
    rhs = S [128 edges, 16 slots] with S[e, slot(row_e)] = edge_val[e].
    8 bins fill a [128 d, 128 slots] PSUM group; a second matmul with W
    projects to [128 slots, 128 d_out]; dma_scatter_add accumulates each
    slot into its destination row of the per-core output buffer (partials
    from different chunks/splits simply add; output starts zeroed).
  - The message path runs in bf16 (X, W, and S values are converted on the
    host): gather payloads are 256B/row and the PE uses fast weight loads.
    PSUM accumulation and the final output stay f32.
  - SWDGE descriptor generation runs on the Q7 core pair selected by
    queue_num, so gathers/scatters are spread over all 4 queues to use 8
    Q7 cores instead of 2.
  - All per-core variability lives in input data; the program is SPMD.
"""

import os
from contextlib import ExitStack

import ml_dtypes
import numpy as np

import concourse.bacc as bacc
import concourse.bass as bass
import concourse.mybir as mybir
import concourse.tile as tile
from concourse import library_config
from concourse.bass_utils import run_bass_kernel_spmd

BF16 = ml_dtypes.bfloat16

N_CORES = 8
D = 128
# dma_gather indices are int16 (< 32768). Use 4 balanced chunks instead of
# 3 full + 1 tiny: a near-empty chunk packs terribly (slot-capped bins).
CHUNK = 25000

# Packing geometry.
R_SLOTS = 16  # row slots per bin
BINS_PER_GROUP = 8  # 8 bins * 16 slots = 128 PSUM slots per group
GROUPS_PER_BATCH = 8
BINS_PER_BATCH = BINS_PER_GROUP * GROUPS_PER_BATCH  # 64
EDGES_PER_BIN = 128
EDGES_PER_BATCH = BINS_PER_BATCH * EDGES_PER_BIN  # 8192

# aux tensor layout (bytes per partition): gather idx | S values | scatter idx
GIDX_B = EDGES_PER_BATCH // 16 * 2  # 1024 ([128, 512] int16)
SVAL_B = BINS_PER_BATCH * R_SLOTS * 2  # 2048 ([128, 1024] bf16)
SIDX_B = GROUPS_PER_BATCH * 128 // 16 * 2  # 128 ([128, 64] int16)
AUX_BYTES = GIDX_B + SVAL_B + SIDX_B  # 3200

last_results = None


def _pack_chunk(r: np.ndarray):
    """Pack row-sorted edges into bins of <=128 edges and <=R_SLOTS rows.
    Rows are NEVER split across bins: each destination row appears at most
    once in this chunk's scatter stream, so concurrent scatter-add
    descriptors never target the same output row (RMW race).
    Returns (ebin, eslot, nbins, slot_rows[nbins, R_SLOTS])."""
    ne = len(r)
    if ne == 0:
        return (
            np.empty(0, np.int64),
            np.empty(0, np.int64),
            0,
            np.empty((0, R_SLOTS), np.int64),
        )
    rows_u, deg = np.unique(r, return_counts=True)
    nrows = len(rows_u)
    assert deg.max() <= EDGES_PER_BIN, "row degree exceeds bin capacity"
    deg_l = deg.tolist()
    # Greedy fill with bounded lookahead: when the next row overflows the
    # bin, pull in a smaller row from the next <=128 rows instead of closing
    # the bin. Bounded displacement keeps each row's scatter position close
    # to its sorted order (cross-chunk scatter race safety).
    LOOK = 128
    used = np.zeros(nrows, bool)
    rbin = np.empty(nrows, np.int64)
    rslot = np.empty(nrows, np.int64)
    s = 0
    nbins = 0
    placed = 0
    while placed < nrows:
        while s < nrows and used[s]:
            s += 1
        if s >= nrows:
            break
        rem = EDGES_PER_BIN
        slot = 0
        j = s
        lim = min(s + LOOK, nrows)
        while slot < R_SLOTS and j < lim:
            if not used[j] and deg_l[j] <= rem:
                used[j] = True
                rbin[j] = nbins
                rslot[j] = slot
                slot += 1
                rem -= deg_l[j]
                placed += 1
                if rem == 0:
                    break
            j += 1
        nbins += 1
    slot_rows = np.full((nbins, R_SLOTS), -1, np.int64)
    slot_rows[rbin, rslot] = rows_u
    # per-edge assignment (r is sorted, so searchsorted maps edge -> row idx)
    ridx = np.searchsorted(rows_u, r)
    return rbin[ridx], rslot[ridx], nbins, slot_rows


def _build_chunk_aux(
    cols_local: np.ndarray,
    vals: np.ndarray,
    ebin: np.ndarray,
    eslot: np.ndarray,
    nbins: int,
    slot_rows: np.ndarray,
    nbatch: int,
    trash_row: int,
):
    ne = len(cols_local)
    bpb, bpg, rs = BINS_PER_BATCH, BINS_PER_GROUP, R_SLOTS
    aux = np.zeros((nbatch, 128, AUX_BYTES), np.int8)
    gidx = aux[:, :, :GIDX_B].view(np.int16)  # [nbatch, 128, 512]
    sval = aux[:, :, GIDX_B : GIDX_B + SVAL_B].view(BF16)  # [nbatch, 128, 1024]
    sidx = aux[:, :, GIDX_B + SVAL_B :].view(np.int16)  # [nbatch, 128, 64]

    if ne:
        # lookahead packing makes per-edge bin ids non-monotone; sort by bin
        order_e = np.argsort(ebin, kind="stable")
        ebin = ebin[order_e]
        eslot = eslot[order_e]
        cols_local = cols_local[order_e]
        vals = vals[order_e]
        starts = np.searchsorted(ebin, np.arange(nbins + 1))
        pos = np.arange(ne) - starts[ebin]
        jb64 = ebin % bpb  # bin within batch
        bb = ebin // bpb  # batch
        # one dma_gather per group of 8 bins (1024 tokens, ucode limit)
        g = jb64 // bpg  # gather/group within batch
        ii = (jb64 % bpg) * EDGES_PER_BIN + pos  # token within gather
        gidx[bb, ii % 16, g * (EDGES_PER_BIN * bpg // 16) + ii // 16] = (
            cols_local.astype(np.int16)
        )
        sval[bb, pos, jb64 * rs + eslot] = vals.astype(BF16)

    # scatter tokens: token t = q*128 + p; bin j (within batch) = q*bpg + p//rs
    jb = np.arange(nbins)
    q = (jb % bpb) // bpg
    base_p = (jb % bpg) * rs
    tok = q[:, None] * 128 + base_p[:, None] + np.arange(rs)[None, :]  # [nbins, rs]
    rows = np.where(slot_rows < 0, trash_row, slot_rows).astype(np.int16)
    b2 = (jb // bpb)[:, None].repeat(rs, 1)
    sidx[:] = trash_row
    sidx[b2, tok % 16, tok // 16] = rows

    # replicate the 16-partition int16 index blocks across all 128 partitions
    gidx[:, 16:, :] = np.tile(gidx[:, :16, :], (1, 7, 1))
    sidx[:, 16:, :] = np.tile(sidx[:, :16, :], (1, 7, 1))
    return aux


def _build_program(n_out: int, nbatches: list[int], gmax: list[int]):
    """gmax[c]: number of live 1024-token gathers (8-bin groups) in chunk c;
    trailing all-padding gathers/groups of the last batch are not emitted."""
    f32 = mybir.dt.float32
    bf16 = mybir.dt.bfloat16
    i16 = mybir.dt.int16
    i8 = mybir.dt.int8
    d = D
    bpg, gpb, rs = BINS_PER_GROUP, GROUPS_PER_BATCH, R_SLOTS
    n_chunks = len(nbatches)

    nc = bacc.Bacc("TRN2", target_bir_lowering=False, num_swdge_queues=4)
    xts = [
        nc.dram_tensor(f"xt{c}", [CHUNK, d], bf16, kind="ExternalInput")
        for c in range(n_chunks)
    ]
    w = nc.dram_tensor("w", [d, d], bf16, kind="ExternalInput")
    auxs = [
        nc.dram_tensor(
            f"aux{c}", [max(nb, 1), 128, AUX_BYTES], i8, kind="ExternalInput"
        )
        for c, nb in enumerate(nbatches)
    ]
    out = nc.dram_tensor("out", [n_out, d], f32, kind="ExternalOutput")

    with ExitStack() as ctx:
        tc = ctx.enter_context(tile.TileContext(nc))
        wpool = ctx.enter_context(tc.tile_pool(name="w", bufs=1))
        msgp = ctx.enter_context(tc.tile_pool(name="msg", bufs=4))
        auxp = ctx.enter_context(tc.tile_pool(name="aux", bufs=6))
        apool = ctx.enter_context(tc.tile_pool(name="aggT", bufs=4))
        bpool = ctx.enter_context(tc.tile_pool(name="outT", bufs=3))
        pa = ctx.enter_context(tc.tile_pool(name="psumA", bufs=2, space="PSUM"))
        pb = ctx.enter_context(tc.tile_pool(name="psumB", bufs=2, space="PSUM"))
        scrp = ctx.enter_context(tc.tile_pool(name="scr", bufs=1, space="PSUM"))

        # PE "absorber" micro-matmuls: the fused LDW+matmul ISA slot only
        # carries one semaphore wait; have PE observe each DMA completion
        # via a 1x1 matmul before the real matmuls.
        scr = scrp.tile([1, 1], f32)

        def absorb(ap_corner):
            nc.tensor.matmul(
                out=scr[:], lhsT=ap_corner, rhs=ap_corner, start=True, stop=True
            )

        nc.gpsimd.load_library(library_config.mlp)
        wt = wpool.tile([d, d], bf16)
        nc.sync.dma_start(wt[:], w[:, :])
        absorb(wt[0:1, 0:1])

        gq = 0  # round-robin gather queue counter
        sq = 0  # scatter queue counter
        for c in range(n_chunks):
            for b in range(nbatches[c]):
                auxt = auxp.tile([128, AUX_BYTES], i8)
                # aux loads go through the idle Activation HWDGE; the SP
                # (sync) sequencer is saturated with semaphore waits.
                nc.scalar.dma_start(auxt[:], auxs[c][b])
                git = auxt[:, 0:GIDX_B].bitcast(i16)
                st = auxt[:, GIDX_B : GIDX_B + SVAL_B].bitcast(bf16)
                sit = auxt[:, GIDX_B + SVAL_B : AUX_BYTES].bitcast(i16)

                n_live = min(gpb, gmax[c] - b * gpb)  # live groups this batch
                msg = msgp.tile([128, BINS_PER_BATCH * d], bf16)
                epg = EDGES_PER_BIN * bpg  # 1024 tokens per gather (ucode max)
                for q in range(n_live):
                    msg3 = msg[:, q * epg : (q + 1) * epg].rearrange(
                        "p (m e) -> p m e", e=d
                    )
                    # Spread gathers over the 4 SWDGE queues: each queue's
                    # descriptor generation runs on a different Q7 core pair
                    # (ucode: cpu_id/2 == queue_num), so they overlap on HW.
                    nc.gpsimd.dma_gather(
                        msg3,
                        xts[c][:, :],
                        git[:, q * (epg // 16) : (q + 1) * (epg // 16)],
                        epg,
                        epg,
                        d,
                        queue_num=gq % 4,
                    )
                    gq += 1
                absorb(st[0:1, 0:1])
                outt = bpool.tile([128, gpb * d], f32)
                for q in range(n_live):
                    pat = pa.tile([128, 128], f32)
                    for jj in range(bpg):
                        binb = q * bpg + jj
                        nc.tensor.matmul(
                            out=pat[:, jj * rs : (jj + 1) * rs],
                            lhsT=msg[:, binb * d : (binb + 1) * d],
                            rhs=st[:, binb * rs : (binb + 1) * rs],
                            start=True,
                            stop=True,
                        )
                    at = apool.tile([128, 128], bf16)
                    nc.vector.tensor_copy(at[:], pat[:])
                    pbt = pb.tile([128, d], f32)
                    nc.tensor.matmul(
                        out=pbt[:], lhsT=at[:], rhs=wt[:], start=True, stop=True
                    )
                    nc.vector.tensor_copy(outt[:, q * d : (q + 1) * d], pbt[:])
                outt3 = outt[:, : n_live * d].rearrange("p (m e) -> p m e", e=d)
                nc.gpsimd.dma_scatter_add(
                    out[:, :],
                    outt3,
                    sit[:, : n_live * 128 // 16],
                    n_live * 128,
                    n_live * 128,
                    d,
                    queue_num=sq % 4,
                )
                sq += 1
    nc.compile()
    return nc


def _prepare(X, W, edge_val, edge_row, edge_col, n_nodes, n_cores=N_CORES):
    """Host-side preprocessing. Returns (nc, in_maps, rows_per_core)."""
    n_nodes = int(n_nodes)
    assert n_nodes % n_cores == 0
    rpc = n_nodes // n_cores
    n_chunks = -(-n_nodes // CHUNK)

    X = np.ascontiguousarray(X, np.float32)
    W = np.ascontiguousarray(W, np.float32)
    edge_val = np.asarray(edge_val, np.float32)
    edge_row = np.asarray(edge_row)
    edge_col = np.asarray(edge_col)

    Xp = np.zeros((n_chunks * CHUNK, D), np.float32)
    Xp[:n_nodes] = X
    xchunks = [
        np.ascontiguousarray(Xp[c * CHUNK : (c + 1) * CHUNK].astype(BF16))
        for c in range(n_chunks)
    ]
    Wb = np.ascontiguousarray(W.astype(BF16))

    chunk_of = edge_col // CHUNK
    order = np.lexsort((edge_row, chunk_of, edge_row // rpc))
    er = edge_row[order]
    ec = edge_col[order]
    ev = edge_val[order]
    ech = chunk_of[order]

    # per (core, chunk) slices
    core_of = er // rpc
    key = core_of * n_chunks + ech
    bounds = np.searchsorted(key, np.arange(n_cores * n_chunks + 1))

    packs = {}
    nb_max = [0] * n_chunks
    gmax = [0] * n_chunks  # live 8-bin gather groups per chunk
    for cidx in range(n_cores):
        for ch in range(n_chunks):
            s, e = bounds[cidx * n_chunks + ch], bounds[cidx * n_chunks + ch + 1]
            lr = er[s:e] - cidx * rpc
            lc = ec[s:e] - ch * CHUNK
            ebin, eslot, nbins, slot_rows = _pack_chunk(lr)
            packs[(cidx, ch)] = (lc, ev[s:e], ebin, eslot, nbins, slot_rows)
            nb_max[ch] = max(nb_max[ch], -(-nbins // BINS_PER_BATCH))
            gmax[ch] = max(gmax[ch], -(-nbins // BINS_PER_GROUP))

    in_maps = []
    for cidx in range(n_cores):
        m = {"w": Wb}
        for ch in range(n_chunks):
            m[f"xt{ch}"] = xchunks[ch]
            lc, vv, ebin, eslot, nbins, slot_rows = packs[(cidx, ch)]
            m[f"aux{ch}"] = _build_chunk_aux(
                lc, vv, ebin, eslot, nbins, slot_rows, max(nb_max[ch], 1), rpc
            )
        in_maps.append(m)

    nc = _build_program(rpc + 8, nb_max, gmax)
    return nc, in_maps, rpc


def kernel(X, W, edge_val, edge_row, edge_col, n_nodes):
    global last_results
    n_nodes = int(n_nodes)
    nc, in_maps, rpc = _prepare(X, W, edge_val, edge_row, edge_col, n_nodes)
    trace = bool(int(os.environ.get("GCN_TRACE", "0")))
    res = run_bass_kernel_spmd(
        nc, in_maps, core_ids=list(range(N_CORES)), trace=trace
    )
    last_results = res
    out = np.concatenate(
        [res.results[c]["out"][:rpc] for c in range(N_CORES)], axis=0
    )
    return out.astype(np.float32)
